# revision 38
# baseline (speedup 1.0000x reference)
"""ChildSum TreeLSTM (B=64 trees, N=512 nodes, D=300) on 8 NeuronCores.

Strategy: data-parallel over trees (8 trees/core). Within a core, nodes are
level-scheduled by height ("waves"); nodes are packed wave-major (sorted by
parent position within each wave) into 128-slot chunks, so child-sum
aggregation becomes small dense matmuls against host-built one-hot selection
blocks.  All matmul traffic is bf16 (PSUM accumulation in fp32).

Key structure:
  - per-gate z PSUM banks (z0/z1/z2/zg) so consecutive windows' matmul /
    activation phases pipeline without whole-tile WAR serialization.
  - partial windows (wl < 128) do not stream the full weight matrices;
    their gate pre-activations are computed up-front in dense "batch"
    windows over compacted tail nodes and injected into PSUM per window
    with host-built shifted-identity matmuls (contraction over batch
    slots), cutting the tail x-side stream from 4500 to ~1500 columns.
  - resident state STc is padded to a 128-wide free dim so the gather
    LDWEIGHTS hits the FWL fast path; inject/transpose stationaries are
    likewise 128 columns.
  - the fc gather is emitted after the hidden matmuls so the PE does not
    head-of-line block on the previous wave's late fst result.
  - state memsets only cover partial-window chunks (whole-chunk, split
    across vector/gpsimd) since full chunks are fully written before read.
"""

import hashlib
import numpy as np
import ml_dtypes

BF16 = ml_dtypes.bfloat16

D = 300
DC = 100          # d-chunk (3 chunks of 100 partitions)
NCORES = 8
P = 128


# ----------------------------------------------------------------- schedule

class _Sched:
    pass


def _build_schedule(parent):
    """parent: [B, N] int array, parent[b,t] in (t, N]; N = sentinel."""
    B, N = parent.shape
    tpc = B // NCORES

    heights = np.zeros((B, N), np.int32)
    for b in range(B):
        h = np.zeros(N + 1, np.int32)
        pb = parent[b]
        for t in range(N):
            ht = h[t] + 1
            p = pb[t]
            if ht > h[p]:
                h[p] = ht
        heights[b] = h[:N]

    Hs = [int(heights[c * tpc:(c + 1) * tpc].max()) + 1 for c in range(NCORES)]
    H = max(Hs)

    sizes = np.zeros((NCORES, H), np.int64)
    for c in range(NCORES):
        cnt = np.bincount(heights[c * tpc:(c + 1) * tpc].ravel(), minlength=H)
        sizes[c] = cnt
    env_real = sizes.max(0)                     # real envelope size per wave
    c_env = ((env_real + P - 1) // P) * P       # 128-padded for ST addressing
    off = np.zeros(H + 1, np.int64)
    off[1:] = np.cumsum(c_env)
    P_total = int(off[H])
    NCH = (P_total + P - 1) // P

    # per-core packing: waves descending so parent positions exist first
    pos_all = np.full((NCORES, tpc, N), -1, np.int64)
    BIG = np.iinfo(np.int64).max
    for c in range(NCORES):
        w = heights[c * tpc:(c + 1) * tpc]
        pb = parent[c * tpc:(c + 1) * tpc]
        pos = pos_all[c]
        for v in range(H - 1, -1, -1):
            bs, ts = np.nonzero(w == v)
            if len(bs) == 0:
                continue
            pp = np.empty(len(bs), np.int64)
            for i in range(len(bs)):
                p = pb[bs[i], ts[i]]
                pp[i] = pos[bs[i], p] if p < N else BIG
            order = np.argsort(pp, kind="stable")
            pos[bs[order], ts[order]] = off[v] + np.arange(len(bs))

    # parent packed position per packed slot (-1 = sentinel parent or padding)
    parr = np.full((NCORES, NCH * P), -1, np.int64)
    for c in range(NCORES):
        pb = parent[c * tpc:(c + 1) * tpc]
        pos = pos_all[c]
        for b in range(tpc):
            for t in range(N):
                p = pb[b, t]
                parr[c, pos[b, t]] = pos[b, p] if p < N else -1

    # windows: one per 128-chunk; wl = envelope-real width (<= 128)
    windows = []  # (v, start, wl)
    for v in range(H):
        s = int(off[v])
        rem = int(env_real[v])
        while rem > 0:
            wl = min(P, rem)
            windows.append((v, s, wl))
            s += P
            rem -= wl

    # ---- inject batching: windows with wl < 128 get their x-side gate
    # pre-activations from dense batch chunks (computed up-front), injected
    # via identity-slice matmuls.  Pack each window's batch range so it never
    # crosses a 128 boundary (one inject segment per window).
    inj = {}      # wi -> (bchunk, lo)
    bc, lo = 0, 0
    for wi, (v, s, wl) in enumerate(windows):
        if wl >= P:
            continue
        if lo + wl > P:
            bc += 1
            lo = 0
        inj[wi] = (bc, lo)
        lo += wl
    NBT = bc + 1 if inj else 0
    inj_slot = {wi: i for i, wi in enumerate(inj)}
    NI = max(1, len(inj))

    # wave of each chunk (waves are 128-padded so chunks don't span waves)
    wave_of_chunk = np.zeros(NCH, np.int64)
    for v in range(H):
        wave_of_chunk[off[v] // P:off[v + 1] // P] = v
    win_of_chunk = {}
    for wi, (v, s, wl) in enumerate(windows):
        win_of_chunk[s // P] = wi

    # selection blocks per window: ST chunks containing any child (any core).
    # Deep-tail windows (one window per wave, wl <= 70) split their blocks:
    # "direct" = chunks of wave v-1 (gathered at window time), "eager" =
    # older chunks, pre-gathered into a spare PSUM bank during the previous
    # window so the serial tail does not pay for the scatter.
    blocks_by_window = []   # direct blocks: list of list of kc
    eager_by_window = []    # eager blocks: list of list of kc
    for wi, (v, s, wl) in enumerate(windows):
        blks, eblks = [], []
        if v > 0:
            chunks = set()
            for c in range(NCORES):
                childpos = np.nonzero((parr[c] >= s) & (parr[c] < s + wl))[0]
                chunks.update((childpos // P).tolist())
            eager_ok = False
            for kc in sorted(chunks):
                if eager_ok and wave_of_chunk[kc] < v - 1:
                    eblks.append(kc)
                else:
                    blks.append(kc)
        blocks_by_window.append(blks)
        eager_by_window.append(eblks)

    sc = _Sched()
    sc.B, sc.N, sc.tpc, sc.H = B, N, tpc, H
    sc.env_real, sc.c_env, sc.off = env_real, c_env, off
    sc.P_total, sc.NCH = P_total, NCH
    sc.pos_all, sc.parr = pos_all, parr
    sc.windows = windows
    sc.inj, sc.NBT = inj, NBT
    sc.inj_slot, sc.NI = inj_slot, NI
    sc.ndir = sum(1 for wi in range(len(windows)) if wi not in inj)
    # direct windows get a slot in the transposed-x image
    sc.dir_slot = {}
    k = 0
    for wi in range(len(windows)):
        if wi not in inj:
            sc.dir_slot[wi] = k
            k += 1
    sc.blocks_by_window = blocks_by_window
    sc.eager_by_window = eager_by_window
    sc.MAXBLK = max(1, max((len(b) for b in blocks_by_window), default=1))
    sc.MAXEBLK = max(1, max((len(b) for b in eager_by_window), default=1))
    # flat offsets in the packed sel stream: direct runs, then eager runs
    sc.selw_off = {}
    run = 0
    for wi, blks in enumerate(blocks_by_window):
        sc.selw_off[wi] = run
        run += len(blks)
    sc.selw_eoff = {}
    for wi, blks in enumerate(eager_by_window):
        sc.selw_eoff[wi] = run
        run += len(blks)
    sc.NB = max(1, run)
    return sc


def _build_core_inputs(sc, c, embs, parent):
    """Per-core input arrays (weights are shared, added separately)."""
    tpc, N, NCH = sc.tpc, sc.N, sc.NCH
    pos = sc.pos_all[c]
    pa = NCH * P

    # packed node -> (b_local, t)
    node_b = np.full(pa, -1, np.int64)
    node_t = np.full(pa, -1, np.int64)
    bs, ts = np.nonzero(pos >= 0)
    node_b[pos[bs, ts]] = bs
    node_t[pos[bs, ts]] = ts

    emb_c = embs[c * tpc:(c + 1) * tpc]  # [tpc, N, D]
    x_rows = np.zeros((pa, D), np.float32)
    real = node_b >= 0
    x_rows[real] = emb_c[node_b[real], node_t[real]]

    pb = parent[c * tpc:(c + 1) * tpc]
    xp_rows = np.zeros((pa, D), np.float32)
    pvals = np.where(real, pb[np.maximum(node_b, 0), np.maximum(node_t, 0)], N)
    has_par = real & (pvals < N)
    xp_rows[has_par] = emb_c[node_b[has_par], pvals[has_par]]

    def tr_block(xb, xpb, wl):
        # [128, 2, 3, 128] transposed x / xp (bias row 1.0 at partition DC)
        out = np.zeros((P, 2, 3, P), BF16)
        for r in range(3):
            out[:DC, 0, r, :wl] = xb[:, r * DC:(r + 1) * DC].T
            out[:DC, 1, r, :wl] = xpb[:, r * DC:(r + 1) * DC].T
        out[DC, 0, 2, :wl] = 1.0
        out[DC, 1, 2, :wl] = 1.0
        return out

    # node-major x rows, [128, NCH, 300] so one DMA loads them all
    xr = np.zeros((P, NCH, D), BF16)
    # transposed x/xp for direct windows
    xtr = np.zeros((max(1, sc.ndir), P, 2, 3, P), BF16)
    # transposed x/xp for inject batch chunks (compacted tail nodes)
    xbat = np.zeros((max(1, sc.NBT), P, 2, 3, P), BF16)
    xbat_acc = [np.zeros((P, D), np.float32) for _ in range(max(1, sc.NBT))]
    xbat_accp = [np.zeros((P, D), np.float32) for _ in range(max(1, sc.NBT))]

    for wi, (v, s, wl) in enumerate(sc.windows):
        ch = s // P
        xb = x_rows[s:s + wl]
        xpb = xp_rows[s:s + wl]
        xr[s % P:s % P + wl, ch] = xb.astype(BF16)
        if wi in sc.inj:
            bc, lo = sc.inj[wi]
            xbat_acc[bc][lo:lo + wl] = xb
            xbat_accp[bc][lo:lo + wl] = xpb
        else:
            xtr[sc.dir_slot[wi]] = tr_block(xb.astype(BF16), xpb.astype(BF16), wl)
    for bc in range(sc.NBT):
        xbat[bc] = tr_block(xbat_acc[bc].astype(BF16),
                            xbat_accp[bc].astype(BF16), P)

    # selection blocks, packed per window in SBUF image order:
    # direct runs first (window order), then eager runs
    sel = np.zeros((sc.NB, P, P), BF16)
    parr_c = sc.parr[c]

    def fill_run(wi, blks, o):
        if not blks:
            return
        nblk = len(blks)
        v, s, wl = sc.windows[wi]
        arr = np.zeros((P, nblk, P), BF16)
        kc2bi = {kc: bi for bi, kc in enumerate(blks)}
        childpos = np.nonzero((parr_c >= s) & (parr_c < s + wl))[0]
        for p in childpos:
            kc = int(p // P)
            if kc in kc2bi:
                arr[int(p % P), kc2bi[kc], parr_c[p] - s] = 1.0
        sel[o:o + nblk] = arr.reshape(nblk, P, P)

    for wi in range(len(sc.windows)):
        fill_run(wi, sc.blocks_by_window[wi], sc.selw_off[wi])
        fill_run(wi, sc.eager_by_window[wi], sc.selw_eoff[wi])

    injsel = np.zeros((P, sc.NI, P), BF16)
    for wi, (bc, lo) in sc.inj.items():
        wl = sc.windows[wi][2]
        sl = sc.inj_slot[wi]
        for i in range(wl):
            injsel[lo + i, sl, i] = 1.0

    return {
        "xr": xr,
        "xtr": xtr,
        "xbat": xbat,
        "sel": sel,
        "injsel": injsel,
    }


def _shared_weights(Wx, bx, Wh, bh, Wt, bt):
    def chunked_x(Wmat, bias):
        # Wmat: [300, M] -> [128, 3, M] with bias row in chunk 2 (partition
        # dim padded to 128 so the load spreads across DMA queues)
        M = Wmat.shape[1]
        out = np.zeros((P, 3, M), np.float32)
        for r in range(3):
            out[:DC, r] = Wmat[r * DC:(r + 1) * DC]
        out[DC, 2] = bias
        return out.astype(BF16)

    def chunked_h(Wmat):
        M = Wmat.shape[1]
        out = np.zeros((P, 3, M), np.float32)
        for r in range(3):
            out[:DC, r] = Wmat[r * DC:(r + 1) * DC]
        return out.astype(BF16)

    wx_iou = np.concatenate([Wx[0], Wx[1], Wx[2]], axis=1)  # [300, 900]
    wh_iou = np.concatenate([Wh[0], Wh[1], Wh[2]], axis=1)
    b_iou = np.concatenate([bx[0] + bh[0], bx[1] + bh[1], bx[2] + bh[2]])
    return {
        "wioux": chunked_x(wx_iou, b_iou),
        "wiouh": chunked_h(wh_iou),
        "wfx": chunked_x(Wx[3], bx[3] + bh[3]),
        "wfh": chunked_h(Wh[3]),
        "wtt": chunked_x(Wt, bt),
    }


# -------------------------------------------------------------- bass module

# flat-column layout of the Z psum tile [P, 1536] (3 banks):
ZIOU0 = 0          # iou cols 0:512     (bank 0)
ZIOU1 = 512        # iou cols 512:900   (bank 1)
ZG = 1024          # g cols 1024:1324   (bank 2)
# zxs (pre-activation stash for inject windows) columns:
XIOU = 0           # 0:900
XG = 900           # 900:1200
XF = 1200          # 1200:1500


def _build_bass(sc):
    import concourse.mybir as mybir
    import concourse.tile as tile
    from concourse import bacc
    from concourse.masks import make_identity

    f32 = mybir.dt.float32
    bf16 = mybir.dt.bfloat16
    AF = mybir.ActivationFunctionType
    OP = mybir.AluOpType

    NCH, NB, H = sc.NCH, sc.NB, sc.H
    MAXBLK = sc.MAXBLK
    MAXEBLK = sc.MAXEBLK
    NBT = max(1, sc.NBT)

    nc = bacc.Bacc()
    xr_d = nc.dram_tensor("xr", [P, NCH, D], bf16, kind="ExternalInput")
    xtr_d = nc.dram_tensor("xtr", [max(1, sc.ndir), P, 2 * 3 * P], bf16,
                           kind="ExternalInput")
    xbat_d = nc.dram_tensor("xbat", [NBT, P, 2 * 3 * P], bf16,
                            kind="ExternalInput")
    sel_d = nc.dram_tensor("sel", [NB, P, P], bf16, kind="ExternalInput")
    injsel_d = nc.dram_tensor("injsel", [P, sc.NI, P], bf16,
                              kind="ExternalInput")
    wioux_d = nc.dram_tensor("wioux", [P, 3, 3 * D], bf16, kind="ExternalInput")
    wiouh_d = nc.dram_tensor("wiouh", [P, 3, 3 * D], bf16, kind="ExternalInput")
    wfx_d = nc.dram_tensor("wfx", [P, 3, D], bf16, kind="ExternalInput")
    wfh_d = nc.dram_tensor("wfh", [P, 3, D], bf16, kind="ExternalInput")
    wtt_d = nc.dram_tensor("wtt", [P, 3, D], bf16, kind="ExternalInput")
    out_d = nc.dram_tensor("out", [NCH, P, D], bf16, kind="ExternalOutput")

    with tile.TileContext(nc) as tc:
        with (
            tc.tile_pool(name="const", bufs=1) as constp,
            tc.tile_pool(name="stp", bufs=1) as stp,
            tc.tile_pool(name="stream", bufs=6) as streamp,
            tc.tile_pool(name="ew", bufs=3) as ewp,
            tc.tile_pool(name="esl", bufs=2) as eslp,
            tc.tile_pool(name="batp", bufs=1) as batp,
            tc.tile_pool(name="ps", bufs=1, space="PSUM") as psp,
        ):
            ident = constp.tile([P, P], bf16)
            make_identity(nc, ident[:])
            injt = constp.tile([P, sc.NI, P], bf16)
            nc.sync.dma_start(injt[:], injsel_d[:])

            xwbs = []
            for bc in range(sc.NBT):
                xwb = batp.tile([P, 2 * 3 * P], bf16, tag=f"xwb{bc}")
                nc.sync.dma_start(xwb[:], xbat_d[bc])
                xwbs.append(xwb)

            wioux = constp.tile([P, 3, 3 * D], bf16)
            nc.sync.dma_start(wioux[:], wioux_d[:])
            wiouh = constp.tile([P, 3, 3 * D], bf16)
            nc.sync.dma_start(wiouh[:], wiouh_d[:])
            wfx = constp.tile([P, 3, D], bf16)
            nc.sync.dma_start(wfx[:], wfx_d[:])
            wfh = constp.tile([P, 3, D], bf16)
            nc.sync.dma_start(wfh[:], wfh_d[:])
            wtt = constp.tile([P, 3, D], bf16)
            nc.sync.dma_start(wtt[:], wtt_d[:])
            # resident packed state, one tile per 128-slot chunk:
            # [128 slots, 6, 100] = st(300) | fst(300)
            # free dim padded to 128 so gather LDWEIGHTS hits the FWL
            # fast path (NumWeights==128)
            STc = [stp.tile([P, 6, P], bf16, name=f"stc{ch}", tag=f"stc{ch}")
                   for ch in range(NCH)]
            # zero only rows that are never written (pad tails of partial
            # windows); real rows are produced before any gather reads them.
            nz = 0
            for wi, (v, s, wl) in enumerate(sc.windows):
                if wl < P:
                    nc.gpsimd.memset(STc[s // P][:, :, :], 0.0)
                    nz += 1
            # pre-activation stash for inject windows (bf16)
            zxs = stp.tile([P, NBT, 1500], bf16, name="zxs", tag="zxs")
            # eager straggler-gather accumulators (per deep-tail window):
            # [:, 0:3, :] hsumT-layout [feat, 3, pos<=DC? no: pos along DC..]
            eaccs = {}
            for wi2 in range(len(sc.windows)):
                if sc.eager_by_window[wi2]:
                    eaccs[wi2] = stp.tile([P, 6, DC], bf16,
                                          name=f"eacc{wi2}", tag=f"eacc{wi2}")
            # node-major x rows, resident (one DMA, issued after the batch
            # loads since it is first consumed late in window 0)
            XR = stp.tile([P, NCH, D], bf16, name="xrs", tag="xrs")

            # PSUM tiles (per-gate z banks decouple the window pipeline)
            z0 = psp.tile([P, 384], f32, tag="z0", name="z0")
            z1 = psp.tile([P, 384], f32, tag="z1", name="z1")
            z2 = psp.tile([P, 384], f32, tag="z2", name="z2")
            zg = psp.tile([P, 384], f32, tag="zg", name="zg")
            f_ps = psp.tile([P, 384], f32, tag="f", name="f")
            fc = psp.tile([P, 384], f32, tag="fc", name="fc")
            hs = psp.tile([P, 3, P], f32, tag="hs", name="hs")
            tp = psp.tile([P, 3, P], bf16, tag="tp", name="tp")
            zb = [z0, z1, z2]

            # ---------------- phase 0: batch windows for inject tail ------
            # (loads issued up-front; matmuls emitted lazily so wave-0
            # windows keep the PE busy while these land)
            def emit_batch():
                for bc in range(sc.NBT):
                    xwb = xwbs[bc]
                    for gi in range(3):
                        for k in range(3):
                            nc.tensor.matmul(
                                zb[gi][:, 0:D],
                                lhsT=xwb[0:DC + 1, k * P:(k + 1) * P],
                                rhs=wioux[0:DC + 1, k, gi * D:(gi + 1) * D],
                                start=(k == 0), stop=(k == 2))
                    for k in range(3):
                        nc.tensor.matmul(
                            zg[:, 0:D],
                            lhsT=xwb[0:DC + 1, k * P:(k + 1) * P],
                            rhs=wtt[0:DC + 1, k, :],
                            start=(k == 0), stop=(k == 2))
                    for k in range(3):
                        nc.tensor.matmul(
                            f_ps[:, 0:D],
                            lhsT=xwb[0:DC + 1, (3 + k) * P:(4 + k) * P],
                            rhs=wfx[0:DC + 1, k, :],
                            start=(k == 0), stop=(k == 2))
                    # stash pre-activations (bf16)
                    for gi in range(3):
                        nc.vector.tensor_copy(
                            zxs[:, bc, XIOU + gi * D:XIOU + (gi + 1) * D],
                            zb[gi][:, 0:D])
                    nc.scalar.copy(zxs[:, bc, XG:XG + D], zg[:, 0:D])
                    nc.scalar.copy(zxs[:, bc, XF:XF + D], f_ps[:, 0:D])

            nc.sync.dma_start(XR[:], xr_d[:])
            first_inj = 0 if sc.inj else -1
            batch_done = [sc.NBT == 0]

            # ---------------- recurrence over windows ----------------------
            for wi, (v, s, wl) in enumerate(sc.windows):
                if not batch_done[0] and wi == first_inj:
                    emit_batch()
                    batch_done[0] = True
                ch = s // P
                blks = sc.blocks_by_window[wi]
                nblk = len(blks)
                last_wave = (v == H - 1)
                is_inj = wi in sc.inj

                if not is_inj:
                    xv = streamp.tile([P, 2 * 3 * P], bf16, tag="xw")
                    nc.sync.dma_start(xv[:], xtr_d[sc.dir_slot[wi]])

                hsumT = None
                has_eager = len(sc.eager_by_window[wi]) > 0
                if v > 0:
                    selt = streamp.tile([P, MAXBLK, P], bf16, tag="sel")
                    o = sc.selw_off[wi]
                    nc.sync.dma_start(selt[:, 0:nblk, :], sel_d[o:o + nblk])
                    # hsumT[f, p] = sum_child st[child, f]
                    hsumT = ewp.tile([DC, 3, P], bf16, tag="hsumT")
                    for r in range(3):
                        for bi, kc in enumerate(blks):
                            nc.tensor.matmul(
                                hs[0:P, r, :wl],
                                lhsT=STc[kc][:, r, :],
                                rhs=selt[:, bi, :wl],
                                start=(bi == 0), stop=(bi == nblk - 1))
                    if has_eager:
                        # fold in the pre-gathered straggler children
                        nc.vector.scalar_tensor_tensor(
                            hsumT[:, 0:3, :wl], hs[0:DC, 0:3, :wl], 1.0,
                            eaccs[wi][0:DC, 0:3, :wl], OP.mult, OP.add)
                    else:
                        nc.vector.tensor_copy(hsumT[:, :, :wl],
                                              hs[0:DC, 0:3, :wl])

                # ---- Z pre-activations: x-side (direct or inject) ---------
                stop0 = (v == 0)
                if is_inj:
                    bc, lo = sc.inj[wi]
                    sl = sc.inj_slot[wi]
                    for gi in range(3):
                        nc.tensor.matmul(
                            zb[gi][:, 0:D],
                            lhsT=injt[:, sl, :],
                            rhs=zxs[:, bc, XIOU + gi * D:XIOU + (gi + 1) * D],
                            start=True, stop=stop0)
                    nc.tensor.matmul(
                        zg[:, 0:D],
                        lhsT=injt[:, sl, :],
                        rhs=zxs[:, bc, XG:XG + D],
                        start=True, stop=True)
                else:
                    for gi in range(3):
                        for k in range(3):
                            nc.tensor.matmul(
                                zb[gi][:wl, 0:D],
                                lhsT=xv[0:DC + 1, k * P:k * P + wl],
                                rhs=wioux[0:DC + 1, k, gi * D:(gi + 1) * D],
                                start=(k == 0), stop=(stop0 and k == 2))
                    for k in range(3):
                        nc.tensor.matmul(
                            zg[:wl, 0:D],
                            lhsT=xv[0:DC + 1, k * P:k * P + wl],
                            rhs=wtt[0:DC + 1, k, :],
                            start=(k == 0), stop=(k == 2))

                # ---- hidden side ------------------------------------------
                if v > 0:
                    for gi in range(3):
                        for k in range(3):
                            nc.tensor.matmul(
                                zb[gi][:wl, 0:D],
                                lhsT=hsumT[:, k, :wl],
                                rhs=wiouh[0:DC, k, gi * D:(gi + 1) * D],
                                start=False, stop=(k == 2))

                # fc gather last on the PE: it depends on the previous
                # wave's fst (produced late); hs/x/hidden must not stall on it
                if v > 0:
                    for bi, kc in enumerate(blks):
                        nc.tensor.matmul(
                            fc[:wl, 0:D],
                            lhsT=selt[:, bi, :wl],
                            rhs=STc[kc][:, 3:6, 0:DC],
                            start=(bi == 0), stop=(bi == nblk - 1))

                # ---- activations ------------------------------------------
                i_sb = ewp.tile([P, D], bf16, tag="i_sb")
                nc.scalar.activation(i_sb[:wl], z0[:wl, 0:D], AF.Sigmoid)
                o_sb = ewp.tile([P, D], bf16, tag="o_sb")
                nc.scalar.activation(o_sb[:wl], z1[:wl, 0:D], AF.Sigmoid)
                u_sb = ewp.tile([P, D], bf16, tag="u_sb")
                nc.scalar.activation(u_sb[:wl], z2[:wl, 0:D], AF.Tanh)
                g_sb = ewp.tile([P, D], bf16, tag="g_sb")
                nc.scalar.activation(g_sb[:wl], zg[:wl, 0:D], AF.Tanh)

                # ---- elementwise (split DVE / gpsimd) ---------------------
                t_sb = ewp.tile([P, D], bf16, tag="t_sb")
                nc.vector.tensor_tensor(t_sb[:wl], i_sb[:wl], u_sb[:wl],
                                        OP.mult)
                c_sb = ewp.tile([P, D], f32, tag="c_sb")
                if v > 0 and has_eager:
                    fc2 = ewp.tile([P, D], f32, tag="fc2")
                    nc.vector.scalar_tensor_tensor(
                        fc2[:wl], fc[:wl, 0:D], 1.0, eaccs[wi][:wl, 3:6, :],
                        OP.mult, OP.add)
                    nc.vector.tensor_tensor(c_sb[:wl], t_sb[:wl],
                                            fc2[:wl], OP.add)
                elif v > 0:
                    nc.vector.tensor_tensor(c_sb[:wl], t_sb[:wl],
                                            fc[:wl, 0:D], OP.add)
                else:
                    nc.vector.tensor_copy(c_sb[:wl], t_sb[:wl])
                tc_sb = ewp.tile([P, D], bf16, tag="tc_sb")
                nc.scalar.activation(tc_sb[:wl], c_sb[:wl], AF.Tanh)
                h_sb = ewp.tile([P, D], bf16, tag="h_sb")
                nc.vector.tensor_tensor(h_sb[:wl], o_sb[:wl],
                                        tc_sb[:wl], OP.mult)
                d_sb = ewp.tile([P, D], bf16, tag="d_sb")
                nc.vector.tensor_tensor(d_sb[:wl], h_sb[:wl], XR[:wl, ch, :],
                                        OP.subtract)
                dg_sb = ewp.tile([P, D], bf16, tag="dg_sb")
                nc.vector.tensor_tensor(dg_sb[:wl], d_sb[:wl], g_sb[:wl],
                                        OP.mult)
                nc.vector.tensor_tensor(STc[ch][:wl, 0:3, 0:DC], dg_sb[:wl],
                                        XR[:wl, ch, :], OP.add)
                nc.scalar.dma_start(out_d[ch], STc[ch][:, 0:3, 0:DC])

                if last_wave:
                    continue

                # stT for the f-gate hidden-side matmul
                for r in range(3):
                    nc.tensor.transpose(tp[0:P, r, :wl],
                                        STc[ch][:wl, r, :],
                                        ident[:wl, :wl])
                stT = ewp.tile([DC, 3, P], bf16, tag="stT")
                nc.vector.tensor_copy(stT[:, :, :wl], tp[0:DC, 0:3, :wl])

                # f = sigmoid(xp @ Wxf + st @ Whf + b); fst = f * st
                if is_inj:
                    bc, lo = sc.inj[wi]
                    sl = sc.inj_slot[wi]
                    nc.tensor.matmul(
                        f_ps[:, 0:D],
                        lhsT=injt[:, sl, :],
                        rhs=zxs[:, bc, XF:XF + D],
                        start=True, stop=False)
                else:
                    for k in range(3):
                        nc.tensor.matmul(
                            f_ps[:wl, 0:D],
                            lhsT=xv[0:DC + 1, (3 + k) * P:(3 + k) * P + wl],
                            rhs=wfx[0:DC + 1, k, :],
                            start=(k == 0), stop=False)
                for k in range(3):
                    nc.tensor.matmul(
                        f_ps[:wl, 0:D], lhsT=stT[:, k, :wl],
                        rhs=wfh[0:DC, k, :],
                        start=False, stop=(k == 2))
                f_sb = ewp.tile([P, D], bf16, tag="f_sb")
                nc.scalar.activation(f_sb[:wl], f_ps[:wl, 0:D], AF.Sigmoid)
                nc.vector.tensor_tensor(STc[ch][:wl, 3:6, 0:DC], f_sb[:wl],
                                        STc[ch][:wl, 0:3, 0:DC], OP.mult)

                # pre-gather the NEXT window's straggler children into the
                # spare bank while this window's act/vector phases run, then
                # park the sums in SBUF (the producers are all final by now)
                nw = wi + 1
                if nw < len(sc.windows) and sc.eager_by_window[nw]:
                    eblks = sc.eager_by_window[nw]
                    ne = len(eblks)
                    nwl = sc.windows[nw][2]
                    eselt = eslp.tile([P, MAXEBLK, P], bf16, tag="esel")
                    eo = sc.selw_eoff[nw]
                    nc.sync.dma_start(eselt[:, 0:ne, :], sel_d[eo:eo + ne])
                    nwl2 = (nwl + 1) // 2 * 2
                    for bi, kc in enumerate(eblks):
                        nc.tensor.matmul(
                            eg[:nwl, 0:D],
                            lhsT=eselt[:, bi, :nwl],
                            rhs=STc[kc][:, 3:6, 0:DC],
                            start=(bi == 0), stop=(bi == ne - 1))
                    nc.vector.tensor_copy(eaccs[nw][:nwl, 3:6, :],
                                          eg[:nwl, 0:D])
                    for r in range(3):
                        o2 = 300 + r * nwl2
                        for bi, kc in enumerate(eblks):
                            nc.tensor.matmul(
                                eg[0:DC, o2:o2 + nwl],
                                lhsT=STc[kc][:, r, :],
                                rhs=eselt[:, bi, :nwl],
                                start=(bi == 0), stop=(bi == ne - 1))
                        nc.vector.tensor_copy(eaccs[nw][0:DC, r, :nwl],
                                              eg[0:DC, o2:o2 + nwl])

    nc.compile()
    return nc


# ------------------------------------------------------------------- driver

_CACHE = {}
LAST_RESULT = None


def kernel(embs, Wx, bx, Wh, bh, Wt, bt, parent):
    global LAST_RESULT
    embs = np.asarray(embs, np.float32)
    Wx = np.asarray(Wx, np.float32)
    bx = np.asarray(bx, np.float32)
    Wh = np.asarray(Wh, np.float32)
    bh = np.asarray(bh, np.float32)
    Wt = np.asarray(Wt, np.float32)
    bt = np.asarray(bt, np.float32)
    parent = np.asarray(parent, np.int64)

    key = hashlib.sha256(parent.tobytes()).hexdigest()
    if key in _CACHE:
        sc, nc = _CACHE[key]
    else:
        sc = _build_schedule(parent)
        nc = _build_bass(sc)
        _CACHE[key] = (sc, nc)

    wts = _shared_weights(Wx, bx, Wh, bh, Wt, bt)
    in_maps = []
    for c in range(NCORES):
        m = _build_core_inputs(sc, c, embs, parent)
        m["xtr"] = m["xtr"].reshape(m["xtr"].shape[0], P, 2 * 3 * P)
        m["xbat"] = m["xbat"].reshape(m["xbat"].shape[0], P, 2 * 3 * P)
        m.update(wts)
        in_maps.append(m)

    from concourse.bass_utils import run_bass_kernel_spmd
    res = run_bass_kernel_spmd(nc, in_maps, core_ids=list(range(NCORES)))
    LAST_RESULT = res

    B, N = parent.shape
    tpc = B // NCORES
    S = np.zeros((B, N, D), np.float32)
    for c in range(NCORES):
        flat = np.asarray(res.results[c]["out"]).astype(np.float32)
        flat = flat.reshape(sc.NCH * P, D)
        pos = sc.pos_all[c]
        S[c * tpc:(c + 1) * tpc] = flat[pos.reshape(-1)].reshape(tpc, N, D)
    return S


# revision 39
# speedup vs baseline: 1.0250x; 1.0250x over previous
"""ChildSum TreeLSTM (B=64 trees, N=512 nodes, D=300) on 8 NeuronCores.

Strategy: data-parallel over trees (8 trees/core). Within a core, nodes are
level-scheduled by height ("waves"); nodes are packed wave-major (sorted by
parent position within each wave) into 128-slot chunks, so child-sum
aggregation becomes small dense matmuls against host-built one-hot selection
blocks.  All matmul traffic is bf16 (PSUM accumulation in fp32).

Key structure:
  - per-gate z PSUM banks (z0/z1/z2/zg) so consecutive windows' matmul /
    activation phases pipeline without whole-tile WAR serialization.
  - partial windows (wl < 128) do not stream the full weight matrices;
    their gate pre-activations are computed up-front in dense "batch"
    windows over compacted tail nodes and injected into PSUM per window
    with host-built shifted-identity matmuls (contraction over batch
    slots), cutting the tail x-side stream from 4500 to ~1500 columns.
  - resident state STc is padded to a 128-wide free dim so the gather
    LDWEIGHTS hits the FWL fast path; inject/transpose stationaries are
    likewise 128 columns.
  - the fc gather is emitted after the hidden matmuls so the PE does not
    head-of-line block on the previous wave's late fst result.
  - state memsets only cover partial-window chunks (whole-chunk, split
    across vector/gpsimd) since full chunks are fully written before read.
"""

import hashlib
import numpy as np
import ml_dtypes

BF16 = ml_dtypes.bfloat16

D = 300
DC = 100          # d-chunk (3 chunks of 100 partitions)
NCORES = 8
P = 128


# ----------------------------------------------------------------- schedule

class _Sched:
    pass


def _build_schedule(parent):
    """parent: [B, N] int array, parent[b,t] in (t, N]; N = sentinel."""
    B, N = parent.shape
    tpc = B // NCORES

    heights = np.zeros((B, N), np.int32)
    for b in range(B):
        h = np.zeros(N + 1, np.int32)
        pb = parent[b]
        for t in range(N):
            ht = h[t] + 1
            p = pb[t]
            if ht > h[p]:
                h[p] = ht
        heights[b] = h[:N]

    Hs = [int(heights[c * tpc:(c + 1) * tpc].max()) + 1 for c in range(NCORES)]
    H = max(Hs)

    sizes = np.zeros((NCORES, H), np.int64)
    for c in range(NCORES):
        cnt = np.bincount(heights[c * tpc:(c + 1) * tpc].ravel(), minlength=H)
        sizes[c] = cnt
    env_real = sizes.max(0)                     # real envelope size per wave
    c_env = ((env_real + P - 1) // P) * P       # 128-padded for ST addressing
    off = np.zeros(H + 1, np.int64)
    off[1:] = np.cumsum(c_env)
    P_total = int(off[H])
    NCH = (P_total + P - 1) // P

    # per-core packing: waves descending so parent positions exist first
    pos_all = np.full((NCORES, tpc, N), -1, np.int64)
    BIG = np.iinfo(np.int64).max
    for c in range(NCORES):
        w = heights[c * tpc:(c + 1) * tpc]
        pb = parent[c * tpc:(c + 1) * tpc]
        pos = pos_all[c]
        for v in range(H - 1, -1, -1):
            bs, ts = np.nonzero(w == v)
            if len(bs) == 0:
                continue
            pp = np.empty(len(bs), np.int64)
            for i in range(len(bs)):
                p = pb[bs[i], ts[i]]
                pp[i] = pos[bs[i], p] if p < N else BIG
            order = np.argsort(pp, kind="stable")
            pos[bs[order], ts[order]] = off[v] + np.arange(len(bs))

    # parent packed position per packed slot (-1 = sentinel parent or padding)
    parr = np.full((NCORES, NCH * P), -1, np.int64)
    for c in range(NCORES):
        pb = parent[c * tpc:(c + 1) * tpc]
        pos = pos_all[c]
        for b in range(tpc):
            for t in range(N):
                p = pb[b, t]
                parr[c, pos[b, t]] = pos[b, p] if p < N else -1

    # windows: one per 128-chunk; wl = envelope-real width (<= 128)
    windows = []  # (v, start, wl)
    for v in range(H):
        s = int(off[v])
        rem = int(env_real[v])
        while rem > 0:
            wl = min(P, rem)
            windows.append((v, s, wl))
            s += P
            rem -= wl

    # ---- inject batching: windows with wl < 128 get their x-side gate
    # pre-activations from dense batch chunks (computed up-front), injected
    # via identity-slice matmuls.  Pack each window's batch range so it never
    # crosses a 128 boundary (one inject segment per window).
    inj = {}      # wi -> (bchunk, lo)
    bc, lo = 0, 0
    for wi, (v, s, wl) in enumerate(windows):
        if wl >= P:
            continue
        if lo + wl > P:
            bc += 1
            lo = 0
        inj[wi] = (bc, lo)
        lo += wl
    NBT = bc + 1 if inj else 0
    inj_slot = {wi: i for i, wi in enumerate(inj)}
    NI = max(1, len(inj))

    # wave of each chunk (waves are 128-padded so chunks don't span waves)
    wave_of_chunk = np.zeros(NCH, np.int64)
    for v in range(H):
        wave_of_chunk[off[v] // P:off[v + 1] // P] = v
    win_of_chunk = {}
    for wi, (v, s, wl) in enumerate(windows):
        win_of_chunk[s // P] = wi

    # selection blocks per window: ST chunks containing any child (any core).
    # Deep-tail windows (one window per wave, wl <= 70) split their blocks:
    # "direct" = chunks of wave v-1 (gathered at window time), "eager" =
    # older chunks, pre-gathered into a spare PSUM bank during the previous
    # window so the serial tail does not pay for the scatter.
    blocks_by_window = []   # direct blocks: list of list of kc
    eager_by_window = []    # eager blocks: list of list of kc
    for wi, (v, s, wl) in enumerate(windows):
        blks, eblks = [], []
        if v > 0:
            chunks = set()
            for c in range(NCORES):
                childpos = np.nonzero((parr[c] >= s) & (parr[c] < s + wl))[0]
                chunks.update((childpos // P).tolist())
            eager_ok = False
            for kc in sorted(chunks):
                if eager_ok and wave_of_chunk[kc] < v - 1:
                    eblks.append(kc)
                else:
                    blks.append(kc)
        blocks_by_window.append(blks)
        eager_by_window.append(eblks)

    sc = _Sched()
    sc.B, sc.N, sc.tpc, sc.H = B, N, tpc, H
    sc.env_real, sc.c_env, sc.off = env_real, c_env, off
    sc.P_total, sc.NCH = P_total, NCH
    sc.pos_all, sc.parr = pos_all, parr
    sc.windows = windows
    sc.inj, sc.NBT = inj, NBT
    sc.inj_slot, sc.NI = inj_slot, NI
    sc.ndir = sum(1 for wi in range(len(windows)) if wi not in inj)
    # direct windows get a slot in the transposed-x image
    sc.dir_slot = {}
    k = 0
    for wi in range(len(windows)):
        if wi not in inj:
            sc.dir_slot[wi] = k
            k += 1
    sc.blocks_by_window = blocks_by_window
    sc.eager_by_window = eager_by_window
    sc.MAXBLK = max(1, max((len(b) for b in blocks_by_window), default=1))
    sc.MAXEBLK = max(1, max((len(b) for b in eager_by_window), default=1))
    # flat offsets in the packed sel stream: direct runs, then eager runs
    sc.selw_off = {}
    run = 0
    for wi, blks in enumerate(blocks_by_window):
        sc.selw_off[wi] = run
        run += len(blks)
    sc.selw_eoff = {}
    for wi, blks in enumerate(eager_by_window):
        sc.selw_eoff[wi] = run
        run += len(blks)
    sc.NB = max(1, run)
    return sc


def _build_core_inputs(sc, c, embs, parent):
    """Per-core input arrays (weights are shared, added separately)."""
    tpc, N, NCH = sc.tpc, sc.N, sc.NCH
    pos = sc.pos_all[c]
    pa = NCH * P

    # packed node -> (b_local, t)
    node_b = np.full(pa, -1, np.int64)
    node_t = np.full(pa, -1, np.int64)
    bs, ts = np.nonzero(pos >= 0)
    node_b[pos[bs, ts]] = bs
    node_t[pos[bs, ts]] = ts

    emb_c = embs[c * tpc:(c + 1) * tpc]  # [tpc, N, D]
    x_rows = np.zeros((pa, D), np.float32)
    real = node_b >= 0
    x_rows[real] = emb_c[node_b[real], node_t[real]]

    pb = parent[c * tpc:(c + 1) * tpc]
    xp_rows = np.zeros((pa, D), np.float32)
    pvals = np.where(real, pb[np.maximum(node_b, 0), np.maximum(node_t, 0)], N)
    has_par = real & (pvals < N)
    xp_rows[has_par] = emb_c[node_b[has_par], pvals[has_par]]

    def tr_block(xb, xpb, wl):
        # [128, 2, 3, 128] transposed x / xp (bias row 1.0 at partition DC)
        out = np.zeros((P, 2, 3, P), BF16)
        for r in range(3):
            out[:DC, 0, r, :wl] = xb[:, r * DC:(r + 1) * DC].T
            out[:DC, 1, r, :wl] = xpb[:, r * DC:(r + 1) * DC].T
        out[DC, 0, 2, :wl] = 1.0
        out[DC, 1, 2, :wl] = 1.0
        return out

    # node-major x rows, [128, NCH, 300] so one DMA loads them all
    xr = np.zeros((P, NCH, D), BF16)
    # transposed x/xp for direct windows
    xtr = np.zeros((max(1, sc.ndir), P, 2, 3, P), BF16)
    # transposed x/xp for inject batch chunks (compacted tail nodes)
    xbat = np.zeros((max(1, sc.NBT), P, 2, 3, P), BF16)
    xbat_acc = [np.zeros((P, D), np.float32) for _ in range(max(1, sc.NBT))]
    xbat_accp = [np.zeros((P, D), np.float32) for _ in range(max(1, sc.NBT))]

    for wi, (v, s, wl) in enumerate(sc.windows):
        ch = s // P
        xb = x_rows[s:s + wl]
        xpb = xp_rows[s:s + wl]
        xr[s % P:s % P + wl, ch] = xb.astype(BF16)
        if wi in sc.inj:
            bc, lo = sc.inj[wi]
            xbat_acc[bc][lo:lo + wl] = xb
            xbat_accp[bc][lo:lo + wl] = xpb
        else:
            xtr[sc.dir_slot[wi]] = tr_block(xb.astype(BF16), xpb.astype(BF16), wl)
    for bc in range(sc.NBT):
        xbat[bc] = tr_block(xbat_acc[bc].astype(BF16),
                            xbat_accp[bc].astype(BF16), P)

    # selection blocks, packed per window in SBUF image order:
    # direct runs first (window order), then eager runs
    sel = np.zeros((sc.NB, P, P), BF16)
    parr_c = sc.parr[c]

    def fill_run(wi, blks, o):
        if not blks:
            return
        nblk = len(blks)
        v, s, wl = sc.windows[wi]
        arr = np.zeros((P, nblk, P), BF16)
        kc2bi = {kc: bi for bi, kc in enumerate(blks)}
        childpos = np.nonzero((parr_c >= s) & (parr_c < s + wl))[0]
        for p in childpos:
            kc = int(p // P)
            if kc in kc2bi:
                arr[int(p % P), kc2bi[kc], parr_c[p] - s] = 1.0
        sel[o:o + nblk] = arr.reshape(nblk, P, P)

    for wi in range(len(sc.windows)):
        fill_run(wi, sc.blocks_by_window[wi], sc.selw_off[wi])
        fill_run(wi, sc.eager_by_window[wi], sc.selw_eoff[wi])

    injsel = np.zeros((P, sc.NI, P), BF16)
    for wi, (bc, lo) in sc.inj.items():
        wl = sc.windows[wi][2]
        sl = sc.inj_slot[wi]
        for i in range(wl):
            injsel[lo + i, sl, i] = 1.0

    return {
        "xr": xr,
        "xtr": xtr,
        "xbat": xbat,
        "sel": sel,
        "injsel": injsel,
    }


def _shared_weights(Wx, bx, Wh, bh, Wt, bt):
    def chunked_x(Wmat, bias):
        # Wmat: [300, M] -> [128, 3, M] with bias row in chunk 2 (partition
        # dim padded to 128 so the load spreads across DMA queues)
        M = Wmat.shape[1]
        out = np.zeros((P, 3, M), np.float32)
        for r in range(3):
            out[:DC, r] = Wmat[r * DC:(r + 1) * DC]
        out[DC, 2] = bias
        return out.astype(BF16)

    def chunked_h(Wmat):
        M = Wmat.shape[1]
        out = np.zeros((P, 3, M), np.float32)
        for r in range(3):
            out[:DC, r] = Wmat[r * DC:(r + 1) * DC]
        return out.astype(BF16)

    wx_iou = np.concatenate([Wx[0], Wx[1], Wx[2]], axis=1)  # [300, 900]
    wh_iou = np.concatenate([Wh[0], Wh[1], Wh[2]], axis=1)
    b_iou = np.concatenate([bx[0] + bh[0], bx[1] + bh[1], bx[2] + bh[2]])
    return {
        "wioux": chunked_x(wx_iou, b_iou),
        "wiouh": chunked_h(wh_iou),
        "wfx": chunked_x(Wx[3], bx[3] + bh[3]),
        "wfh": chunked_h(Wh[3]),
        "wtt": chunked_x(Wt, bt),
    }


# -------------------------------------------------------------- bass module

# flat-column layout of the Z psum tile [P, 1536] (3 banks):
ZIOU0 = 0          # iou cols 0:512     (bank 0)
ZIOU1 = 512        # iou cols 512:900   (bank 1)
ZG = 1024          # g cols 1024:1324   (bank 2)
# zxs (pre-activation stash for inject windows) columns:
XIOU = 0           # 0:900
XG = 900           # 900:1200
XF = 1200          # 1200:1500


def _build_bass(sc):
    import concourse.mybir as mybir
    import concourse.tile as tile
    from concourse import bacc
    from concourse.masks import make_identity

    f32 = mybir.dt.float32
    bf16 = mybir.dt.bfloat16
    AF = mybir.ActivationFunctionType
    OP = mybir.AluOpType

    NCH, NB, H = sc.NCH, sc.NB, sc.H
    MAXBLK = sc.MAXBLK
    MAXEBLK = sc.MAXEBLK
    NBT = max(1, sc.NBT)

    nc = bacc.Bacc()
    xr_d = nc.dram_tensor("xr", [P, NCH, D], bf16, kind="ExternalInput")
    xtr_d = nc.dram_tensor("xtr", [max(1, sc.ndir), P, 2 * 3 * P], bf16,
                           kind="ExternalInput")
    xbat_d = nc.dram_tensor("xbat", [NBT, P, 2 * 3 * P], bf16,
                            kind="ExternalInput")
    sel_d = nc.dram_tensor("sel", [NB, P, P], bf16, kind="ExternalInput")
    injsel_d = nc.dram_tensor("injsel", [P, sc.NI, P], bf16,
                              kind="ExternalInput")
    wioux_d = nc.dram_tensor("wioux", [P, 3, 3 * D], bf16, kind="ExternalInput")
    wiouh_d = nc.dram_tensor("wiouh", [P, 3, 3 * D], bf16, kind="ExternalInput")
    wfx_d = nc.dram_tensor("wfx", [P, 3, D], bf16, kind="ExternalInput")
    wfh_d = nc.dram_tensor("wfh", [P, 3, D], bf16, kind="ExternalInput")
    wtt_d = nc.dram_tensor("wtt", [P, 3, D], bf16, kind="ExternalInput")
    out_d = nc.dram_tensor("out", [NCH, P, D], bf16, kind="ExternalOutput")

    with tile.TileContext(nc) as tc:
        with (
            tc.tile_pool(name="const", bufs=1) as constp,
            tc.tile_pool(name="stp", bufs=1) as stp,
            tc.tile_pool(name="stream", bufs=6) as streamp,
            tc.tile_pool(name="ew", bufs=3) as ewp,
            tc.tile_pool(name="esl", bufs=2) as eslp,
            tc.tile_pool(name="batp", bufs=1) as batp,
            tc.tile_pool(name="ps", bufs=1, space="PSUM") as psp,
        ):
            ident = constp.tile([P, P], bf16)
            make_identity(nc, ident[:])
            injt = constp.tile([P, sc.NI, P], bf16)
            nc.sync.dma_start(injt[:], injsel_d[:])

            wioux = constp.tile([P, 3, 3 * D], bf16)
            nc.sync.dma_start(wioux[:], wioux_d[:])
            wiouh = constp.tile([P, 3, 3 * D], bf16)
            nc.sync.dma_start(wiouh[:], wiouh_d[:])
            wfx = constp.tile([P, 3, D], bf16)
            nc.sync.dma_start(wfx[:], wfx_d[:])
            wfh = constp.tile([P, 3, D], bf16)
            nc.sync.dma_start(wfh[:], wfh_d[:])
            wtt = constp.tile([P, 3, D], bf16)
            nc.sync.dma_start(wtt[:], wtt_d[:])
            xwbs = []
            for bc in range(sc.NBT):
                xwb = batp.tile([P, 2 * 3 * P], bf16, tag=f"xwb{bc}")
                nc.sync.dma_start(xwb[:], xbat_d[bc])
                xwbs.append(xwb)

            # resident packed state, one tile per 128-slot chunk:
            # [128 slots, 6, 100] = st(300) | fst(300)
            # free dim padded to 128 so gather LDWEIGHTS hits the FWL
            # fast path (NumWeights==128)
            STc = [stp.tile([P, 6, P], bf16, name=f"stc{ch}", tag=f"stc{ch}")
                   for ch in range(NCH)]
            # zero only rows that are never written (pad tails of partial
            # windows); real rows are produced before any gather reads them.
            nz = 0
            for wi, (v, s, wl) in enumerate(sc.windows):
                if wl < P:
                    eng = nc.vector if nz % 2 == 0 else nc.gpsimd
                    eng.memset(STc[s // P][:, :, :], 0.0)
                    nz += 1
            # pre-activation stash for inject windows (bf16)
            zxs = stp.tile([P, NBT, 1500], bf16, name="zxs", tag="zxs")
            # eager straggler-gather accumulators (per deep-tail window):
            # [:, 0:3, :] hsumT-layout [feat, 3, pos<=DC? no: pos along DC..]
            eaccs = {}
            for wi2 in range(len(sc.windows)):
                if sc.eager_by_window[wi2]:
                    eaccs[wi2] = stp.tile([P, 6, DC], bf16,
                                          name=f"eacc{wi2}", tag=f"eacc{wi2}")
            # node-major x rows, resident (one DMA, issued after the batch
            # loads since it is first consumed late in window 0)
            XR = stp.tile([P, NCH, D], bf16, name="xrs", tag="xrs")

            # PSUM tiles (per-gate z banks decouple the window pipeline)
            z0 = psp.tile([P, 384], f32, tag="z0", name="z0")
            z1 = psp.tile([P, 384], f32, tag="z1", name="z1")
            z2 = psp.tile([P, 384], f32, tag="z2", name="z2")
            zg = psp.tile([P, 384], f32, tag="zg", name="zg")
            f_ps = psp.tile([P, 384], f32, tag="f", name="f")
            fc = psp.tile([P, 384], f32, tag="fc", name="fc")
            hs = psp.tile([P, 3, P], f32, tag="hs", name="hs")
            tp = psp.tile([P, 3, P], bf16, tag="tp", name="tp")
            zb = [z0, z1, z2]

            # ---------------- phase 0: batch windows for inject tail ------
            # (loads issued up-front; matmuls emitted lazily so wave-0
            # windows keep the PE busy while these land)
            def emit_batch():
                for bc in range(sc.NBT):
                    xwb = xwbs[bc]
                    for gi in range(3):
                        for k in range(3):
                            nc.tensor.matmul(
                                zb[gi][:, 0:D],
                                lhsT=xwb[0:DC + 1, k * P:(k + 1) * P],
                                rhs=wioux[0:DC + 1, k, gi * D:(gi + 1) * D],
                                start=(k == 0), stop=(k == 2))
                    for k in range(3):
                        nc.tensor.matmul(
                            zg[:, 0:D],
                            lhsT=xwb[0:DC + 1, k * P:(k + 1) * P],
                            rhs=wtt[0:DC + 1, k, :],
                            start=(k == 0), stop=(k == 2))
                    for k in range(3):
                        nc.tensor.matmul(
                            f_ps[:, 0:D],
                            lhsT=xwb[0:DC + 1, (3 + k) * P:(4 + k) * P],
                            rhs=wfx[0:DC + 1, k, :],
                            start=(k == 0), stop=(k == 2))
                    # stash pre-activations (bf16)
                    for gi in range(3):
                        nc.vector.tensor_copy(
                            zxs[:, bc, XIOU + gi * D:XIOU + (gi + 1) * D],
                            zb[gi][:, 0:D])
                    nc.scalar.copy(zxs[:, bc, XG:XG + D], zg[:, 0:D])
                    nc.scalar.copy(zxs[:, bc, XF:XF + D], f_ps[:, 0:D])

            nc.sync.dma_start(XR[:], xr_d[:])
            first_inj = 0 if sc.inj else -1
            batch_done = [sc.NBT == 0]

            # ---------------- recurrence over windows ----------------------
            for wi, (v, s, wl) in enumerate(sc.windows):
                if not batch_done[0] and wi == first_inj:
                    emit_batch()
                    batch_done[0] = True
                ch = s // P
                blks = sc.blocks_by_window[wi]
                nblk = len(blks)
                last_wave = (v == H - 1)
                is_inj = wi in sc.inj

                if not is_inj:
                    xv = streamp.tile([P, 2 * 3 * P], bf16, tag="xw")
                    nc.sync.dma_start(xv[:], xtr_d[sc.dir_slot[wi]])

                hsumT = None
                has_eager = len(sc.eager_by_window[wi]) > 0
                if v > 0:
                    selt = streamp.tile([P, MAXBLK, P], bf16, tag="sel")
                    o = sc.selw_off[wi]
                    nc.sync.dma_start(selt[:, 0:nblk, :], sel_d[o:o + nblk])
                    # hsumT[f, p] = sum_child st[child, f]
                    hsumT = ewp.tile([DC, 3, P], bf16, tag="hsumT")
                    for r in range(3):
                        for bi, kc in enumerate(blks):
                            nc.tensor.matmul(
                                hs[0:P, r, :wl],
                                lhsT=STc[kc][:, r, :],
                                rhs=selt[:, bi, :wl],
                                start=(bi == 0), stop=(bi == nblk - 1))
                    if has_eager:
                        # fold in the pre-gathered straggler children
                        nc.vector.scalar_tensor_tensor(
                            hsumT[:, 0:3, :wl], hs[0:DC, 0:3, :wl], 1.0,
                            eaccs[wi][0:DC, 0:3, :wl], OP.mult, OP.add)
                    else:
                        nc.vector.tensor_copy(hsumT[:, :, :wl],
                                              hs[0:DC, 0:3, :wl])

                # ---- Z pre-activations: x-side (direct or inject) ---------
                stop0 = (v == 0)
                if is_inj:
                    bc, lo = sc.inj[wi]
                    sl = sc.inj_slot[wi]
                    for gi in range(3):
                        nc.tensor.matmul(
                            zb[gi][:, 0:D],
                            lhsT=injt[:, sl, :],
                            rhs=zxs[:, bc, XIOU + gi * D:XIOU + (gi + 1) * D],
                            start=True, stop=stop0)
                    nc.tensor.matmul(
                        zg[:, 0:D],
                        lhsT=injt[:, sl, :],
                        rhs=zxs[:, bc, XG:XG + D],
                        start=True, stop=True)
                else:
                    for gi in range(3):
                        for k in range(3):
                            nc.tensor.matmul(
                                zb[gi][:wl, 0:D],
                                lhsT=xv[0:DC + 1, k * P:k * P + wl],
                                rhs=wioux[0:DC + 1, k, gi * D:(gi + 1) * D],
                                start=(k == 0), stop=(stop0 and k == 2))
                    for k in range(3):
                        nc.tensor.matmul(
                            zg[:wl, 0:D],
                            lhsT=xv[0:DC + 1, k * P:k * P + wl],
                            rhs=wtt[0:DC + 1, k, :],
                            start=(k == 0), stop=(k == 2))

                # ---- hidden side ------------------------------------------
                if v > 0:
                    for gi in range(3):
                        for k in range(3):
                            nc.tensor.matmul(
                                zb[gi][:wl, 0:D],
                                lhsT=hsumT[:, k, :wl],
                                rhs=wiouh[0:DC, k, gi * D:(gi + 1) * D],
                                start=False, stop=(k == 2))

                # fc gather last on the PE: it depends on the previous
                # wave's fst (produced late); hs/x/hidden must not stall on it
                if v > 0:
                    for bi, kc in enumerate(blks):
                        nc.tensor.matmul(
                            fc[:wl, 0:D],
                            lhsT=selt[:, bi, :wl],
                            rhs=STc[kc][:, 3:6, 0:DC],
                            start=(bi == 0), stop=(bi == nblk - 1))

                # ---- activations ------------------------------------------
                i_sb = ewp.tile([P, D], bf16, tag="i_sb")
                nc.scalar.activation(i_sb[:wl], z0[:wl, 0:D], AF.Sigmoid)
                o_sb = ewp.tile([P, D], bf16, tag="o_sb")
                nc.scalar.activation(o_sb[:wl], z1[:wl, 0:D], AF.Sigmoid)
                u_sb = ewp.tile([P, D], bf16, tag="u_sb")
                nc.scalar.activation(u_sb[:wl], z2[:wl, 0:D], AF.Tanh)
                g_sb = ewp.tile([P, D], bf16, tag="g_sb")
                nc.scalar.activation(g_sb[:wl], zg[:wl, 0:D], AF.Tanh)

                # ---- elementwise (split DVE / gpsimd) ---------------------
                t_sb = ewp.tile([P, D], bf16, tag="t_sb")
                nc.vector.tensor_tensor(t_sb[:wl], i_sb[:wl], u_sb[:wl],
                                        OP.mult)
                c_sb = ewp.tile([P, D], f32, tag="c_sb")
                if v > 0 and has_eager:
                    fc2 = ewp.tile([P, D], f32, tag="fc2")
                    nc.vector.scalar_tensor_tensor(
                        fc2[:wl], fc[:wl, 0:D], 1.0, eaccs[wi][:wl, 3:6, :],
                        OP.mult, OP.add)
                    nc.vector.tensor_tensor(c_sb[:wl], t_sb[:wl],
                                            fc2[:wl], OP.add)
                elif v > 0:
                    nc.vector.tensor_tensor(c_sb[:wl], t_sb[:wl],
                                            fc[:wl, 0:D], OP.add)
                else:
                    nc.vector.tensor_copy(c_sb[:wl], t_sb[:wl])
                tc_sb = ewp.tile([P, D], bf16, tag="tc_sb")
                nc.scalar.activation(tc_sb[:wl], c_sb[:wl], AF.Tanh)
                h_sb = ewp.tile([P, D], bf16, tag="h_sb")
                nc.vector.tensor_tensor(h_sb[:wl], o_sb[:wl],
                                        tc_sb[:wl], OP.mult)
                d_sb = ewp.tile([P, D], bf16, tag="d_sb")
                nc.vector.tensor_tensor(d_sb[:wl], h_sb[:wl], XR[:wl, ch, :],
                                        OP.subtract)
                dg_sb = ewp.tile([P, D], bf16, tag="dg_sb")
                nc.vector.tensor_tensor(dg_sb[:wl], d_sb[:wl], g_sb[:wl],
                                        OP.mult)
                nc.vector.tensor_tensor(STc[ch][:wl, 0:3, 0:DC], dg_sb[:wl],
                                        XR[:wl, ch, :], OP.add)
                nc.scalar.dma_start(out_d[ch], STc[ch][:, 0:3, 0:DC])

                if last_wave:
                    continue

                # stT for the f-gate hidden-side matmul
                for r in range(3):
                    nc.tensor.transpose(tp[0:P, r, :wl],
                                        STc[ch][:wl, r, :],
                                        ident[:wl, :wl])
                stT = ewp.tile([DC, 3, P], bf16, tag="stT")
                nc.vector.tensor_copy(stT[:, :, :wl], tp[0:DC, 0:3, :wl])

                # f = sigmoid(xp @ Wxf + st @ Whf + b); fst = f * st
                if is_inj:
                    bc, lo = sc.inj[wi]
                    sl = sc.inj_slot[wi]
                    nc.tensor.matmul(
                        f_ps[:, 0:D],
                        lhsT=injt[:, sl, :],
                        rhs=zxs[:, bc, XF:XF + D],
                        start=True, stop=False)
                else:
                    for k in range(3):
                        nc.tensor.matmul(
                            f_ps[:wl, 0:D],
                            lhsT=xv[0:DC + 1, (3 + k) * P:(3 + k) * P + wl],
                            rhs=wfx[0:DC + 1, k, :],
                            start=(k == 0), stop=False)
                for k in range(3):
                    nc.tensor.matmul(
                        f_ps[:wl, 0:D], lhsT=stT[:, k, :wl],
                        rhs=wfh[0:DC, k, :],
                        start=False, stop=(k == 2))
                f_sb = ewp.tile([P, D], bf16, tag="f_sb")
                nc.scalar.activation(f_sb[:wl], f_ps[:wl, 0:D], AF.Sigmoid)
                nc.vector.tensor_tensor(STc[ch][:wl, 3:6, 0:DC], f_sb[:wl],
                                        STc[ch][:wl, 0:3, 0:DC], OP.mult)

                # pre-gather the NEXT window's straggler children into the
                # spare bank while this window's act/vector phases run, then
                # park the sums in SBUF (the producers are all final by now)
                nw = wi + 1
                if nw < len(sc.windows) and sc.eager_by_window[nw]:
                    eblks = sc.eager_by_window[nw]
                    ne = len(eblks)
                    nwl = sc.windows[nw][2]
                    eselt = eslp.tile([P, MAXEBLK, P], bf16, tag="esel")
                    eo = sc.selw_eoff[nw]
                    nc.sync.dma_start(eselt[:, 0:ne, :], sel_d[eo:eo + ne])
                    nwl2 = (nwl + 1) // 2 * 2
                    for bi, kc in enumerate(eblks):
                        nc.tensor.matmul(
                            eg[:nwl, 0:D],
                            lhsT=eselt[:, bi, :nwl],
                            rhs=STc[kc][:, 3:6, 0:DC],
                            start=(bi == 0), stop=(bi == ne - 1))
                    nc.vector.tensor_copy(eaccs[nw][:nwl, 3:6, :],
                                          eg[:nwl, 0:D])
                    for r in range(3):
                        o2 = 300 + r * nwl2
                        for bi, kc in enumerate(eblks):
                            nc.tensor.matmul(
                                eg[0:DC, o2:o2 + nwl],
                                lhsT=STc[kc][:, r, :],
                                rhs=eselt[:, bi, :nwl],
                                start=(bi == 0), stop=(bi == ne - 1))
                        nc.vector.tensor_copy(eaccs[nw][0:DC, r, :nwl],
                                              eg[0:DC, o2:o2 + nwl])

    nc.compile()
    return nc


# ------------------------------------------------------------------- driver

_CACHE = {}
LAST_RESULT = None


def kernel(embs, Wx, bx, Wh, bh, Wt, bt, parent):
    global LAST_RESULT
    embs = np.asarray(embs, np.float32)
    Wx = np.asarray(Wx, np.float32)
    bx = np.asarray(bx, np.float32)
    Wh = np.asarray(Wh, np.float32)
    bh = np.asarray(bh, np.float32)
    Wt = np.asarray(Wt, np.float32)
    bt = np.asarray(bt, np.float32)
    parent = np.asarray(parent, np.int64)

    key = hashlib.sha256(parent.tobytes()).hexdigest()
    if key in _CACHE:
        sc, nc = _CACHE[key]
    else:
        sc = _build_schedule(parent)
        nc = _build_bass(sc)
        _CACHE[key] = (sc, nc)

    wts = _shared_weights(Wx, bx, Wh, bh, Wt, bt)
    in_maps = []
    for c in range(NCORES):
        m = _build_core_inputs(sc, c, embs, parent)
        m["xtr"] = m["xtr"].reshape(m["xtr"].shape[0], P, 2 * 3 * P)
        m["xbat"] = m["xbat"].reshape(m["xbat"].shape[0], P, 2 * 3 * P)
        m.update(wts)
        in_maps.append(m)

    from concourse.bass_utils import run_bass_kernel_spmd
    res = run_bass_kernel_spmd(nc, in_maps, core_ids=list(range(NCORES)))
    LAST_RESULT = res

    B, N = parent.shape
    tpc = B // NCORES
    S = np.zeros((B, N, D), np.float32)
    for c in range(NCORES):
        flat = np.asarray(res.results[c]["out"]).astype(np.float32)
        flat = flat.reshape(sc.NCH * P, D)
        pos = sc.pos_all[c]
        S[c * tpc:(c + 1) * tpc] = flat[pos.reshape(-1)].reshape(tpc, N, D)
    return S


# revision 40
# speedup vs baseline: 1.0648x; 1.0389x over previous
"""ChildSum TreeLSTM (B=64 trees, N=512 nodes, D=300) on 8 NeuronCores.

Strategy: data-parallel over trees (8 trees/core). Within a core, nodes are
level-scheduled by height ("waves"); nodes are packed wave-major (sorted by
parent position within each wave) into 128-slot chunks, so child-sum
aggregation becomes small dense matmuls against host-built one-hot selection
blocks.  All matmul traffic is bf16 (PSUM accumulation in fp32).

Key structure:
  - per-gate z PSUM banks (z0/z1/z2/zg) so consecutive windows' matmul /
    activation phases pipeline without whole-tile WAR serialization.
  - partial windows (wl < 128) do not stream the full weight matrices;
    their gate pre-activations are computed up-front in dense "batch"
    windows over compacted tail nodes and injected into PSUM per window
    with host-built shifted-identity matmuls (contraction over batch
    slots), cutting the tail x-side stream from 4500 to ~1500 columns.
  - resident state STc is padded to a 128-wide free dim so the gather
    LDWEIGHTS hits the FWL fast path; inject/transpose stationaries are
    likewise 128 columns.
  - the fc gather is emitted after the hidden matmuls so the PE does not
    head-of-line block on the previous wave's late fst result.
  - state memsets only cover partial-window chunks (whole-chunk, split
    across vector/gpsimd) since full chunks are fully written before read.
"""

import hashlib
import numpy as np
import ml_dtypes

BF16 = ml_dtypes.bfloat16

D = 300
DC = 100          # d-chunk (3 chunks of 100 partitions)
NCORES = 8
P = 128


# ----------------------------------------------------------------- schedule

class _Sched:
    pass


def _build_schedule(parent):
    """parent: [B, N] int array, parent[b,t] in (t, N]; N = sentinel."""
    B, N = parent.shape
    tpc = B // NCORES

    heights = np.zeros((B, N), np.int32)
    for b in range(B):
        h = np.zeros(N + 1, np.int32)
        pb = parent[b]
        for t in range(N):
            ht = h[t] + 1
            p = pb[t]
            if ht > h[p]:
                h[p] = ht
        heights[b] = h[:N]

    Hs = [int(heights[c * tpc:(c + 1) * tpc].max()) + 1 for c in range(NCORES)]
    H = max(Hs)

    sizes = np.zeros((NCORES, H), np.int64)
    for c in range(NCORES):
        cnt = np.bincount(heights[c * tpc:(c + 1) * tpc].ravel(), minlength=H)
        sizes[c] = cnt
    env_real = sizes.max(0)                     # real envelope size per wave
    c_env = ((env_real + P - 1) // P) * P       # 128-padded for ST addressing
    off = np.zeros(H + 1, np.int64)
    off[1:] = np.cumsum(c_env)
    P_total = int(off[H])
    NCH = (P_total + P - 1) // P

    # per-core packing: waves descending so parent positions exist first
    pos_all = np.full((NCORES, tpc, N), -1, np.int64)
    BIG = np.iinfo(np.int64).max
    for c in range(NCORES):
        w = heights[c * tpc:(c + 1) * tpc]
        pb = parent[c * tpc:(c + 1) * tpc]
        pos = pos_all[c]
        for v in range(H - 1, -1, -1):
            bs, ts = np.nonzero(w == v)
            if len(bs) == 0:
                continue
            pp = np.empty(len(bs), np.int64)
            for i in range(len(bs)):
                p = pb[bs[i], ts[i]]
                pp[i] = pos[bs[i], p] if p < N else BIG
            order = np.argsort(pp, kind="stable")
            pos[bs[order], ts[order]] = off[v] + np.arange(len(bs))

    # parent packed position per packed slot (-1 = sentinel parent or padding)
    parr = np.full((NCORES, NCH * P), -1, np.int64)
    for c in range(NCORES):
        pb = parent[c * tpc:(c + 1) * tpc]
        pos = pos_all[c]
        for b in range(tpc):
            for t in range(N):
                p = pb[b, t]
                parr[c, pos[b, t]] = pos[b, p] if p < N else -1

    # windows: one per 128-chunk; wl = envelope-real width (<= 128)
    windows = []  # (v, start, wl)
    for v in range(H):
        s = int(off[v])
        rem = int(env_real[v])
        while rem > 0:
            wl = min(P, rem)
            windows.append((v, s, wl))
            s += P
            rem -= wl

    # ---- inject batching: windows with wl < 128 get their x-side gate
    # pre-activations from dense batch chunks (computed up-front), injected
    # via identity-slice matmuls.  Pack each window's batch range so it never
    # crosses a 128 boundary (one inject segment per window).
    inj = {}      # wi -> (bchunk, lo)
    bc, lo = 0, 0
    for wi, (v, s, wl) in enumerate(windows):
        if wl >= P:
            continue
        if lo + wl > P:
            bc += 1
            lo = 0
        inj[wi] = (bc, lo)
        lo += wl
    NBT = bc + 1 if inj else 0
    inj_slot = {wi: i for i, wi in enumerate(inj)}
    NI = max(1, len(inj))

    # wave of each chunk (waves are 128-padded so chunks don't span waves)
    wave_of_chunk = np.zeros(NCH, np.int64)
    for v in range(H):
        wave_of_chunk[off[v] // P:off[v + 1] // P] = v
    win_of_chunk = {}
    for wi, (v, s, wl) in enumerate(windows):
        win_of_chunk[s // P] = wi

    # selection blocks per window: ST chunks containing any child (any core).
    # Deep-tail windows (one window per wave, wl <= 70) split their blocks:
    # "direct" = chunks of wave v-1 (gathered at window time), "eager" =
    # older chunks, pre-gathered into a spare PSUM bank during the previous
    # window so the serial tail does not pay for the scatter.
    blocks_by_window = []   # direct blocks: list of list of kc
    eager_by_window = []    # eager blocks: list of list of kc
    for wi, (v, s, wl) in enumerate(windows):
        blks, eblks = [], []
        if v > 0:
            chunks = set()
            for c in range(NCORES):
                childpos = np.nonzero((parr[c] >= s) & (parr[c] < s + wl))[0]
                chunks.update((childpos // P).tolist())
            eager_ok = False
            for kc in sorted(chunks):
                if eager_ok and wave_of_chunk[kc] < v - 1:
                    eblks.append(kc)
                else:
                    blks.append(kc)
        blocks_by_window.append(blks)
        eager_by_window.append(eblks)

    sc = _Sched()
    sc.B, sc.N, sc.tpc, sc.H = B, N, tpc, H
    sc.env_real, sc.c_env, sc.off = env_real, c_env, off
    sc.P_total, sc.NCH = P_total, NCH
    sc.pos_all, sc.parr = pos_all, parr
    sc.windows = windows
    sc.inj, sc.NBT = inj, NBT
    sc.inj_slot, sc.NI = inj_slot, NI
    sc.ndir = sum(1 for wi in range(len(windows)) if wi not in inj)
    # direct windows get a slot in the transposed-x image
    sc.dir_slot = {}
    k = 0
    for wi in range(len(windows)):
        if wi not in inj:
            sc.dir_slot[wi] = k
            k += 1
    sc.blocks_by_window = blocks_by_window
    sc.eager_by_window = eager_by_window
    sc.MAXBLK = max(1, max((len(b) for b in blocks_by_window), default=1))
    sc.MAXEBLK = max(1, max((len(b) for b in eager_by_window), default=1))
    # flat offsets in the packed sel stream: direct runs, then eager runs
    sc.selw_off = {}
    run = 0
    for wi, blks in enumerate(blocks_by_window):
        sc.selw_off[wi] = run
        run += len(blks)
    sc.selw_eoff = {}
    for wi, blks in enumerate(eager_by_window):
        sc.selw_eoff[wi] = run
        run += len(blks)
    sc.NB = max(1, run)
    return sc


def _build_core_inputs(sc, c, embs, parent):
    """Per-core input arrays (weights are shared, added separately)."""
    tpc, N, NCH = sc.tpc, sc.N, sc.NCH
    pos = sc.pos_all[c]
    pa = NCH * P

    # packed node -> (b_local, t)
    node_b = np.full(pa, -1, np.int64)
    node_t = np.full(pa, -1, np.int64)
    bs, ts = np.nonzero(pos >= 0)
    node_b[pos[bs, ts]] = bs
    node_t[pos[bs, ts]] = ts

    emb_c = embs[c * tpc:(c + 1) * tpc]  # [tpc, N, D]
    x_rows = np.zeros((pa, D), np.float32)
    real = node_b >= 0
    x_rows[real] = emb_c[node_b[real], node_t[real]]

    pb = parent[c * tpc:(c + 1) * tpc]
    xp_rows = np.zeros((pa, D), np.float32)
    pvals = np.where(real, pb[np.maximum(node_b, 0), np.maximum(node_t, 0)], N)
    has_par = real & (pvals < N)
    xp_rows[has_par] = emb_c[node_b[has_par], pvals[has_par]]

    def tr_block(xb, xpb, wl):
        # [128, 2, 3, 128] transposed x / xp (bias row 1.0 at partition DC)
        out = np.zeros((P, 2, 3, P), BF16)
        for r in range(3):
            out[:DC, 0, r, :wl] = xb[:, r * DC:(r + 1) * DC].T
            out[:DC, 1, r, :wl] = xpb[:, r * DC:(r + 1) * DC].T
        out[DC, 0, 2, :wl] = 1.0
        out[DC, 1, 2, :wl] = 1.0
        return out

    # node-major x rows, [128, NCH, 300] so one DMA loads them all
    xr = np.zeros((P, NCH, D), BF16)
    # transposed x/xp for direct windows
    xtr = np.zeros((max(1, sc.ndir), P, 2, 3, P), BF16)
    # transposed x/xp for inject batch chunks (compacted tail nodes)
    xbat = np.zeros((max(1, sc.NBT), P, 2, 3, P), BF16)
    xbat_acc = [np.zeros((P, D), np.float32) for _ in range(max(1, sc.NBT))]
    xbat_accp = [np.zeros((P, D), np.float32) for _ in range(max(1, sc.NBT))]

    for wi, (v, s, wl) in enumerate(sc.windows):
        ch = s // P
        xb = x_rows[s:s + wl]
        xpb = xp_rows[s:s + wl]
        xr[s % P:s % P + wl, ch] = xb.astype(BF16)
        if wi in sc.inj:
            bc, lo = sc.inj[wi]
            xbat_acc[bc][lo:lo + wl] = xb
            xbat_accp[bc][lo:lo + wl] = xpb
        else:
            xtr[sc.dir_slot[wi]] = tr_block(xb.astype(BF16), xpb.astype(BF16), wl)
    for bc in range(sc.NBT):
        xbat[bc] = tr_block(xbat_acc[bc].astype(BF16),
                            xbat_accp[bc].astype(BF16), P)

    # selection blocks, packed per window in SBUF image order:
    # direct runs first (window order), then eager runs
    sel = np.zeros((sc.NB, P, P), BF16)
    parr_c = sc.parr[c]

    def fill_run(wi, blks, o):
        if not blks:
            return
        nblk = len(blks)
        v, s, wl = sc.windows[wi]
        arr = np.zeros((P, nblk, P), BF16)
        kc2bi = {kc: bi for bi, kc in enumerate(blks)}
        childpos = np.nonzero((parr_c >= s) & (parr_c < s + wl))[0]
        for p in childpos:
            kc = int(p // P)
            if kc in kc2bi:
                arr[int(p % P), kc2bi[kc], parr_c[p] - s] = 1.0
        sel[o:o + nblk] = arr.reshape(nblk, P, P)

    for wi in range(len(sc.windows)):
        fill_run(wi, sc.blocks_by_window[wi], sc.selw_off[wi])
        fill_run(wi, sc.eager_by_window[wi], sc.selw_eoff[wi])

    injsel = np.zeros((P, sc.NI, P), BF16)
    for wi, (bc, lo) in sc.inj.items():
        wl = sc.windows[wi][2]
        sl = sc.inj_slot[wi]
        for i in range(wl):
            injsel[lo + i, sl, i] = 1.0

    return {
        "xr": xr,
        "xtr": xtr,
        "xbat": xbat,
        "sel": sel,
        "injsel": injsel,
    }


def _shared_weights(Wx, bx, Wh, bh, Wt, bt):
    def chunked_x(Wmat, bias):
        # Wmat: [300, M] -> [128, 3, M] with bias row in chunk 2 (partition
        # dim padded to 128 so the load spreads across DMA queues)
        M = Wmat.shape[1]
        out = np.zeros((P, 3, M), np.float32)
        for r in range(3):
            out[:DC, r] = Wmat[r * DC:(r + 1) * DC]
        out[DC, 2] = bias
        return out.astype(BF16)

    def chunked_h(Wmat):
        M = Wmat.shape[1]
        out = np.zeros((P, 3, M), np.float32)
        for r in range(3):
            out[:DC, r] = Wmat[r * DC:(r + 1) * DC]
        return out.astype(BF16)

    wx_iou = np.concatenate([Wx[0], Wx[1], Wx[2]], axis=1)  # [300, 900]
    wh_iou = np.concatenate([Wh[0], Wh[1], Wh[2]], axis=1)
    b_iou = np.concatenate([bx[0] + bh[0], bx[1] + bh[1], bx[2] + bh[2]])
    return {
        "wioux": chunked_x(wx_iou, b_iou),
        "wiouh": chunked_h(wh_iou),
        "wfx": chunked_x(Wx[3], bx[3] + bh[3]),
        "wfh": chunked_h(Wh[3]),
        "wtt": chunked_x(Wt, bt),
    }


# -------------------------------------------------------------- bass module

# flat-column layout of the Z psum tile [P, 1536] (3 banks):
ZIOU0 = 0          # iou cols 0:512     (bank 0)
ZIOU1 = 512        # iou cols 512:900   (bank 1)
ZG = 1024          # g cols 1024:1324   (bank 2)
# zxs (pre-activation stash for inject windows) columns:
XIOU = 0           # 0:900
XG = 900           # 900:1200
XF = 1200          # 1200:1500


def _build_bass(sc):
    import concourse.mybir as mybir
    import concourse.tile as tile
    from concourse import bacc
    from concourse.masks import make_identity

    f32 = mybir.dt.float32
    bf16 = mybir.dt.bfloat16
    AF = mybir.ActivationFunctionType
    OP = mybir.AluOpType

    NCH, NB, H = sc.NCH, sc.NB, sc.H
    MAXBLK = sc.MAXBLK
    MAXEBLK = sc.MAXEBLK
    NBT = max(1, sc.NBT)

    nc = bacc.Bacc()
    xr_d = nc.dram_tensor("xr", [P, NCH, D], bf16, kind="ExternalInput")
    xtr_d = nc.dram_tensor("xtr", [max(1, sc.ndir), P, 2 * 3 * P], bf16,
                           kind="ExternalInput")
    xbat_d = nc.dram_tensor("xbat", [NBT, P, 2 * 3 * P], bf16,
                            kind="ExternalInput")
    sel_d = nc.dram_tensor("sel", [NB, P, P], bf16, kind="ExternalInput")
    injsel_d = nc.dram_tensor("injsel", [P, sc.NI, P], bf16,
                              kind="ExternalInput")
    wioux_d = nc.dram_tensor("wioux", [P, 3, 3 * D], bf16, kind="ExternalInput")
    wiouh_d = nc.dram_tensor("wiouh", [P, 3, 3 * D], bf16, kind="ExternalInput")
    wfx_d = nc.dram_tensor("wfx", [P, 3, D], bf16, kind="ExternalInput")
    wfh_d = nc.dram_tensor("wfh", [P, 3, D], bf16, kind="ExternalInput")
    wtt_d = nc.dram_tensor("wtt", [P, 3, D], bf16, kind="ExternalInput")
    out_d = nc.dram_tensor("out", [NCH, P, D], bf16, kind="ExternalOutput")

    with tile.TileContext(nc) as tc:
        with (
            tc.tile_pool(name="const", bufs=1) as constp,
            tc.tile_pool(name="stp", bufs=1) as stp,
            tc.tile_pool(name="stream", bufs=6) as streamp,
            tc.tile_pool(name="ew", bufs=3) as ewp,
            tc.tile_pool(name="esl", bufs=2) as eslp,
            tc.tile_pool(name="batp", bufs=1) as batp,
            tc.tile_pool(name="ps", bufs=1, space="PSUM") as psp,
        ):
            ident = constp.tile([P, P], bf16)
            make_identity(nc, ident[:])
            injt = constp.tile([P, sc.NI, P], bf16)
            nc.sync.dma_start(injt[:], injsel_d[:])

            wioux = constp.tile([P, 3, 3 * D], bf16)
            nc.sync.dma_start(wioux[:], wioux_d[:])
            wiouh = constp.tile([P, 3, 3 * D], bf16)
            nc.sync.dma_start(wiouh[:], wiouh_d[:])
            wfx = constp.tile([P, 3, D], bf16)
            nc.sync.dma_start(wfx[:], wfx_d[:])
            wfh = constp.tile([P, 3, D], bf16)
            nc.sync.dma_start(wfh[:], wfh_d[:])
            wtt = constp.tile([P, 3, D], bf16)
            nc.sync.dma_start(wtt[:], wtt_d[:])
            xwbs = []
            for bc in range(sc.NBT):
                xwb = batp.tile([P, 2 * 3 * P], bf16, tag=f"xwb{bc}")
                nc.sync.dma_start(xwb[:], xbat_d[bc])
                xwbs.append(xwb)

            # resident packed state, one tile per 128-slot chunk:
            # [128 slots, 6, 100] = st(300) | fst(300)
            # free dim padded to 128 so gather LDWEIGHTS hits the FWL
            # fast path (NumWeights==128)
            STc = [stp.tile([P, 6, P], bf16, name=f"stc{ch}", tag=f"stc{ch}")
                   for ch in range(NCH)]
            # zero only rows that are never written (pad tails of partial
            # windows); real rows are produced before any gather reads them.
            nz = 0
            for wi, (v, s, wl) in enumerate(sc.windows):
                if wl < P:
                    eng = nc.vector if nz % 2 == 0 else nc.gpsimd
                    eng.memset(STc[s // P][:, :, :], 0.0)
                    nz += 1
            # pre-activation stash for inject windows (bf16)
            zxs = stp.tile([P, NBT, 1500], bf16, name="zxs", tag="zxs")
            # eager straggler-gather accumulators (per deep-tail window):
            # [:, 0:3, :] hsumT-layout [feat, 3, pos<=DC? no: pos along DC..]
            eaccs = {}
            for wi2 in range(len(sc.windows)):
                if sc.eager_by_window[wi2]:
                    eaccs[wi2] = stp.tile([P, 6, DC], bf16,
                                          name=f"eacc{wi2}", tag=f"eacc{wi2}")
            # node-major x rows, resident (one DMA, issued after the batch
            # loads since it is first consumed late in window 0)
            XR = stp.tile([P, NCH, D], bf16, name="xrs", tag="xrs")

            # PSUM tiles (per-gate z banks decouple the window pipeline)
            z0 = psp.tile([P, 384], f32, tag="z0", name="z0")
            z1 = psp.tile([P, 384], f32, tag="z1", name="z1")
            z2 = psp.tile([P, 384], f32, tag="z2", name="z2")
            zg = psp.tile([P, 384], f32, tag="zg", name="zg")
            f_ps = psp.tile([P, 384], f32, tag="f", name="f")
            fc = psp.tile([P, 384], f32, tag="fc", name="fc")
            hs = psp.tile([P, 3, P], f32, tag="hs", name="hs")
            tp = psp.tile([P, 3, P], bf16, tag="tp", name="tp")
            zb = [z0, z1, z2]

            # ---------------- phase 0: batch windows for inject tail ------
            # (loads issued up-front; matmuls emitted lazily so wave-0
            # windows keep the PE busy while these land)
            def emit_batch():
                for bc in range(sc.NBT):
                    xwb = xwbs[bc]
                    for gi in range(3):
                        for k in range(3):
                            nc.tensor.matmul(
                                zb[gi][:, 0:D],
                                lhsT=xwb[0:DC + 1, k * P:(k + 1) * P],
                                rhs=wioux[0:DC + 1, k, gi * D:(gi + 1) * D],
                                start=(k == 0), stop=(k == 2))
                    for k in range(3):
                        nc.tensor.matmul(
                            zg[:, 0:D],
                            lhsT=xwb[0:DC + 1, k * P:(k + 1) * P],
                            rhs=wtt[0:DC + 1, k, :],
                            start=(k == 0), stop=(k == 2))
                    for k in range(3):
                        nc.tensor.matmul(
                            f_ps[:, 0:D],
                            lhsT=xwb[0:DC + 1, (3 + k) * P:(4 + k) * P],
                            rhs=wfx[0:DC + 1, k, :],
                            start=(k == 0), stop=(k == 2))
                    # stash pre-activations (bf16)
                    for gi in range(3):
                        nc.vector.tensor_copy(
                            zxs[:, bc, XIOU + gi * D:XIOU + (gi + 1) * D],
                            zb[gi][:, 0:D])
                    nc.scalar.copy(zxs[:, bc, XG:XG + D], zg[:, 0:D])
                    nc.scalar.copy(zxs[:, bc, XF:XF + D], f_ps[:, 0:D])

            nc.sync.dma_start(XR[:], xr_d[:])
            first_inj = 0 if sc.inj else -1
            batch_done = [sc.NBT == 0]

            # ---------------- recurrence over windows ----------------------
            for wi, (v, s, wl) in enumerate(sc.windows):
                if not batch_done[0] and wi == first_inj:
                    emit_batch()
                    batch_done[0] = True
                ch = s // P
                blks = sc.blocks_by_window[wi]
                nblk = len(blks)
                last_wave = (v == H - 1)
                is_inj = wi in sc.inj

                if not is_inj:
                    xv = streamp.tile([P, 2 * 3 * P], bf16, tag="xw")
                    nc.sync.dma_start(xv[:], xtr_d[sc.dir_slot[wi]])

                hsumT = None
                has_eager = len(sc.eager_by_window[wi]) > 0
                if v > 0:
                    selt = streamp.tile([P, MAXBLK, P], bf16, tag="sel")
                    o = sc.selw_off[wi]
                    nc.sync.dma_start(selt[:, 0:nblk, :], sel_d[o:o + nblk])
                    # hsumT[f, p] = sum_child st[child, f]
                    hsumT = ewp.tile([DC, 3, P], bf16, tag="hsumT")
                    for r in range(3):
                        for bi, kc in enumerate(blks):
                            nc.tensor.matmul(
                                hs[0:P, r, :wl],
                                lhsT=STc[kc][:, r, :],
                                rhs=selt[:, bi, :wl],
                                start=(bi == 0), stop=(bi == nblk - 1))
                    if has_eager:
                        # fold in the pre-gathered straggler children
                        nc.vector.scalar_tensor_tensor(
                            hsumT[:, 0:3, :wl], hs[0:DC, 0:3, :wl], 1.0,
                            eaccs[wi][0:DC, 0:3, :wl], OP.mult, OP.add)
                    else:
                        nc.vector.tensor_copy(hsumT[:, :, :wl],
                                              hs[0:DC, 0:3, :wl])

                # ---- Z pre-activations: x-side (direct or inject) ---------
                stop0 = (v == 0)
                if is_inj:
                    bc, lo = sc.inj[wi]
                    sl = sc.inj_slot[wi]
                    for gi in range(3):
                        nc.tensor.matmul(
                            zb[gi][:, 0:D],
                            lhsT=injt[:, sl, :],
                            rhs=zxs[:, bc, XIOU + gi * D:XIOU + (gi + 1) * D],
                            start=True, stop=stop0)
                    nc.tensor.matmul(
                        zg[:, 0:D],
                        lhsT=injt[:, sl, :],
                        rhs=zxs[:, bc, XG:XG + D],
                        start=True, stop=True)
                else:
                    for gi in range(3):
                        for k in range(3):
                            nc.tensor.matmul(
                                zb[gi][:wl, 0:D],
                                lhsT=xv[0:DC + 1, k * P:k * P + wl],
                                rhs=wioux[0:DC + 1, k, gi * D:(gi + 1) * D],
                                start=(k == 0), stop=(stop0 and k == 2))
                    for k in range(3):
                        nc.tensor.matmul(
                            zg[:wl, 0:D],
                            lhsT=xv[0:DC + 1, k * P:k * P + wl],
                            rhs=wtt[0:DC + 1, k, :],
                            start=(k == 0), stop=(k == 2))

                # ---- hidden side ------------------------------------------
                if v > 0:
                    for gi in range(3):
                        for k in range(3):
                            nc.tensor.matmul(
                                zb[gi][:wl, 0:D],
                                lhsT=hsumT[:, k, :wl],
                                rhs=wiouh[0:DC, k, gi * D:(gi + 1) * D],
                                start=False, stop=(k == 2))

                # fc gather last on the PE: it depends on the previous
                # wave's fst (produced late); hs/x/hidden must not stall on it
                if v > 0:
                    for bi, kc in enumerate(blks):
                        nc.tensor.matmul(
                            fc[:wl, 0:D],
                            lhsT=selt[:, bi, :wl],
                            rhs=STc[kc][:, 3:6, 0:DC],
                            start=(bi == 0), stop=(bi == nblk - 1))

                # ---- activations ------------------------------------------
                i_sb = ewp.tile([P, D], bf16, tag="i_sb")
                nc.scalar.activation(i_sb[:wl], z0[:wl, 0:D], AF.Sigmoid)
                o_sb = ewp.tile([P, D], bf16, tag="o_sb")
                nc.scalar.activation(o_sb[:wl], z1[:wl, 0:D], AF.Sigmoid)
                u_sb = ewp.tile([P, D], bf16, tag="u_sb")
                nc.scalar.activation(u_sb[:wl], z2[:wl, 0:D], AF.Tanh)
                g_sb = ewp.tile([P, D], bf16, tag="g_sb")
                nc.scalar.activation(g_sb[:wl], zg[:wl, 0:D], AF.Tanh)

                # ---- elementwise (split DVE / gpsimd) ---------------------
                t_sb = ewp.tile([P, D], bf16, tag="t_sb")
                nc.vector.tensor_tensor(t_sb[:wl], i_sb[:wl], u_sb[:wl],
                                        OP.mult)
                c_sb = ewp.tile([P, D], f32, tag="c_sb")
                if v > 0 and has_eager:
                    fc2 = ewp.tile([P, D], f32, tag="fc2")
                    nc.vector.scalar_tensor_tensor(
                        fc2[:wl], fc[:wl, 0:D], 1.0, eaccs[wi][:wl, 3:6, :],
                        OP.mult, OP.add)
                    nc.vector.tensor_tensor(c_sb[:wl], t_sb[:wl],
                                            fc2[:wl], OP.add)
                elif v > 0:
                    nc.vector.tensor_tensor(c_sb[:wl], t_sb[:wl],
                                            fc[:wl, 0:D], OP.add)
                else:
                    nc.vector.tensor_copy(c_sb[:wl], t_sb[:wl])
                tc_sb = ewp.tile([P, D], bf16, tag="tc_sb")
                nc.scalar.activation(tc_sb[:wl], c_sb[:wl], AF.Tanh)
                h_sb = ewp.tile([P, D], bf16, tag="h_sb")
                nc.vector.tensor_tensor(h_sb[:wl], o_sb[:wl],
                                        tc_sb[:wl], OP.mult)
                d_sb = ewp.tile([P, D], bf16, tag="d_sb")
                nc.vector.tensor_tensor(d_sb[:wl], h_sb[:wl], XR[:wl, ch, :],
                                        OP.subtract)
                dg_sb = ewp.tile([P, D], bf16, tag="dg_sb")
                nc.vector.tensor_tensor(dg_sb[:wl], d_sb[:wl], g_sb[:wl],
                                        OP.mult)
                nc.vector.tensor_tensor(STc[ch][:wl, 0:3, 0:DC], dg_sb[:wl],
                                        XR[:wl, ch, :], OP.add)
                nc.sync.dma_start(out_d[ch], STc[ch][:, 0:3, 0:DC])

                if last_wave:
                    continue

                # stT for the f-gate hidden-side matmul
                for r in range(3):
                    nc.tensor.transpose(tp[0:P, r, :wl],
                                        STc[ch][:wl, r, :],
                                        ident[:wl, :wl])
                stT = ewp.tile([DC, 3, P], bf16, tag="stT")
                nc.vector.tensor_copy(stT[:, :, :wl], tp[0:DC, 0:3, :wl])

                # f = sigmoid(xp @ Wxf + st @ Whf + b); fst = f * st
                if is_inj:
                    bc, lo = sc.inj[wi]
                    sl = sc.inj_slot[wi]
                    nc.tensor.matmul(
                        f_ps[:, 0:D],
                        lhsT=injt[:, sl, :],
                        rhs=zxs[:, bc, XF:XF + D],
                        start=True, stop=False)
                else:
                    for k in range(3):
                        nc.tensor.matmul(
                            f_ps[:wl, 0:D],
                            lhsT=xv[0:DC + 1, (3 + k) * P:(3 + k) * P + wl],
                            rhs=wfx[0:DC + 1, k, :],
                            start=(k == 0), stop=False)
                for k in range(3):
                    nc.tensor.matmul(
                        f_ps[:wl, 0:D], lhsT=stT[:, k, :wl],
                        rhs=wfh[0:DC, k, :],
                        start=False, stop=(k == 2))
                f_sb = ewp.tile([P, D], bf16, tag="f_sb")
                nc.scalar.activation(f_sb[:wl], f_ps[:wl, 0:D], AF.Sigmoid)
                nc.vector.tensor_tensor(STc[ch][:wl, 3:6, 0:DC], f_sb[:wl],
                                        STc[ch][:wl, 0:3, 0:DC], OP.mult)

                # pre-gather the NEXT window's straggler children into the
                # spare bank while this window's act/vector phases run, then
                # park the sums in SBUF (the producers are all final by now)
                nw = wi + 1
                if nw < len(sc.windows) and sc.eager_by_window[nw]:
                    eblks = sc.eager_by_window[nw]
                    ne = len(eblks)
                    nwl = sc.windows[nw][2]
                    eselt = eslp.tile([P, MAXEBLK, P], bf16, tag="esel")
                    eo = sc.selw_eoff[nw]
                    nc.sync.dma_start(eselt[:, 0:ne, :], sel_d[eo:eo + ne])
                    nwl2 = (nwl + 1) // 2 * 2
                    for bi, kc in enumerate(eblks):
                        nc.tensor.matmul(
                            eg[:nwl, 0:D],
                            lhsT=eselt[:, bi, :nwl],
                            rhs=STc[kc][:, 3:6, 0:DC],
                            start=(bi == 0), stop=(bi == ne - 1))
                    nc.vector.tensor_copy(eaccs[nw][:nwl, 3:6, :],
                                          eg[:nwl, 0:D])
                    for r in range(3):
                        o2 = 300 + r * nwl2
                        for bi, kc in enumerate(eblks):
                            nc.tensor.matmul(
                                eg[0:DC, o2:o2 + nwl],
                                lhsT=STc[kc][:, r, :],
                                rhs=eselt[:, bi, :nwl],
                                start=(bi == 0), stop=(bi == ne - 1))
                        nc.vector.tensor_copy(eaccs[nw][0:DC, r, :nwl],
                                              eg[0:DC, o2:o2 + nwl])

    nc.compile()
    return nc


# ------------------------------------------------------------------- driver

_CACHE = {}
LAST_RESULT = None


def kernel(embs, Wx, bx, Wh, bh, Wt, bt, parent):
    global LAST_RESULT
    embs = np.asarray(embs, np.float32)
    Wx = np.asarray(Wx, np.float32)
    bx = np.asarray(bx, np.float32)
    Wh = np.asarray(Wh, np.float32)
    bh = np.asarray(bh, np.float32)
    Wt = np.asarray(Wt, np.float32)
    bt = np.asarray(bt, np.float32)
    parent = np.asarray(parent, np.int64)

    key = hashlib.sha256(parent.tobytes()).hexdigest()
    if key in _CACHE:
        sc, nc = _CACHE[key]
    else:
        sc = _build_schedule(parent)
        nc = _build_bass(sc)
        _CACHE[key] = (sc, nc)

    wts = _shared_weights(Wx, bx, Wh, bh, Wt, bt)
    in_maps = []
    for c in range(NCORES):
        m = _build_core_inputs(sc, c, embs, parent)
        m["xtr"] = m["xtr"].reshape(m["xtr"].shape[0], P, 2 * 3 * P)
        m["xbat"] = m["xbat"].reshape(m["xbat"].shape[0], P, 2 * 3 * P)
        m.update(wts)
        in_maps.append(m)

    from concourse.bass_utils import run_bass_kernel_spmd
    res = run_bass_kernel_spmd(nc, in_maps, core_ids=list(range(NCORES)))
    LAST_RESULT = res

    B, N = parent.shape
    tpc = B // NCORES
    S = np.zeros((B, N, D), np.float32)
    for c in range(NCORES):
        flat = np.asarray(res.results[c]["out"]).astype(np.float32)
        flat = flat.reshape(sc.NCH * P, D)
        pos = sc.pos_all[c]
        S[c * tpc:(c + 1) * tpc] = flat[pos.reshape(-1)].reshape(tpc, N, D)
    return S
